# revision 44
# baseline (speedup 1.0000x reference)
"""Trainium2 Bass kernel for nn_Attention_68685116997866.

Math (per batch b; C=128, N=32768):
    A = q_w @ y + q_b,  K = k_w @ x + k_b          (pointwise convs)
    energy = [A;K] @ [A;K]^T / sqrt(2C)            ([256,256] Gram)
    e1 = relu(energy @ t1_w^T + t1_b)
    e2 = relu(e1 @ t2_w^T + t2_b)
    attn = softmax(e2, axis=-1)                    ([256,128])
    out  = (attn_top^T @ v2_w) @ y + (attn_bot^T @ v1_w) @ x
         + (attn_top^T @ v2_b + attn_bot^T @ v1_b) 1^T

Strategy: data-parallel over B across 8 cores (1 batch/core), no
collectives. Inputs ship as fp8-e4m3 RESIDUAL PAIRS — zh = e4m3(z),
zl = e4m3(16*(z - zh)) — the same 2 bytes/elem as bf16 (phase 2 needs
2-byte fidelity; pure-fp8 z there fails the 2e-2 gate at 2.9e-2), but
split into two separate streams so each phase reads only what it needs:

  Phase 1 (Gram) starts as soon as the hi stream flows: fp8
  PE-transposes (output elem-step 2, a HW rule) into per-pair PSUM
  banks, one DVE/ACT-alternated copy per 2 groups into fp8 T-pair
  tiles [128,2,2,272] (DR k-tile stride must be 16-aligned; GPSIMD
  cannot read PSUM so Pool only handles SBUF memsets), then fp8
  DoubleRow matmuls (K=256, 0.5 cyc/col): S_top full width, S_bot
  right half only (symmetry), ones columns for the col-sums. Only the
  first matmul of the epoch sets start=True: start zeroes the whole
  2KB PSUM bank. The lo stream DMAs during phase-1 compute.

  Postlude: energy = W S W^T + v c^T + c(v + n c)^T via U = S W^T;
  the E stage is folded into MLP1 on the host (TWt = (t1w W / s)^T)
  with rank-1 bias terms joining MLP1's PSUM accumulation. Postlude
  matmuls are f32r (1 cyc/col at >=256 moving cols; even col offsets
  and even free sizes required).

  Phase 2 = three DoubleRow matmuls per 512-col chunk, scale-matched
  into one PSUM group: Wa*zh + Wb*zl + Wc*zh with Wa = e4m3(128 W),
  Wb = e4m3(8 W), Wc = e4m3(128 W - Wa) (e4m3 max 240: 128*W is safe,
  larger scales overflow when softmax concentrates). The stage op
  applies *(1/128) + bout and out-DMAs are merged into 2048-col quads
  (the HWDGE issue port costs ~650ns per DMA). PSUM rotates through
  the 3 pp banks plus the retired gram bank.

Segments pack [y | x] per segment so each needs ONE DMA; hi segments
ramp 512->4096 then back down so the gram tail is short.

rel err 1.48e-2 vs fp32 reference (gram-fp8 attn shift 1.38e-2 is the
dominant term; hi/lo phase 2 and bf16 out add the rest).
"""

import sys

for _p in ("/opt/trn_rl_repo",):
    if _p not in sys.path:
        sys.path.insert(0, _p)

import numpy as np
import ml_dtypes

import concourse.bass as bass  # noqa: F401
import concourse.mybir as mybir
import concourse.tile as tile
from concourse import bacc
from concourse.bass_utils import run_bass_kernel_spmd

B, C, N = 8, 128, 32768
F32 = mybir.dt.float32
F32R = mybir.dt.float32r
BF16 = mybir.dt.bfloat16
FP8 = mybir.dt.float8e4
AF = mybir.ActivationFunctionType
DR = mybir.MatmulPerfMode.DoubleRow

WSCALE = 128.0  # Wa scale; Wb = WSCALE/16, lo-plane carries 16*(z-zh)


def _seg_sizes_h(n):
    if n == N:
        return [512, 1024, 2048, 4096, 4096, 4096, 4096, 4096, 4096,
                2048, 1024, 1024, 512]
    segs = 4
    return [n // segs] * segs


def _seg_sizes_l(n):
    if n == N:
        return [4096, 4096, 4096, 4096, 4096, 4096, 4096, 2048, 1024,
                512, 512]
    segs = 4
    return [n // segs] * segs


def build_program(n=N, repeat=1, lag=6, ntp=8, nppb=4,
                  skip_phase2=False, skip_gram=False):
    """Build the per-core Bass program (one batch per core)."""
    nc = bacc.Bacc(None, target_bir_lowering=False)
    hsizes = _seg_sizes_h(n)
    lsizes = _seg_sizes_l(n)
    hstarts = np.concatenate([[0], np.cumsum(hsizes)]).tolist()
    lstarts = np.concatenate([[0], np.cumsum(lsizes)]).tolist()
    n_chunks = n // 128
    assert n_chunks % 2 == 0

    def locate(col, starts, nseg):
        for s in range(nseg):
            if col < starts[s + 1]:
                return s, col - starts[s]
        raise AssertionError(col)

    oc = 512
    out_chunks = n // oc

    # ---- DRAM I/O ----
    # hb/lb pack y and x per segment: [yseg_s | xseg_s] blocks.
    hb_d = nc.dram_tensor("hb", [128, 2 * n], FP8, kind="ExternalInput")
    lb_d = nc.dram_tensor("lb", [128, 2 * n], FP8, kind="ExternalInput")
    bblob_d = nc.dram_tensor("bblob", [128, 128], FP8, kind="ExternalInput")
    fblob_d = nc.dram_tensor("fblob", [128, 1672], F32R, kind="ExternalInput")
    rblob_d = nc.dram_tensor("rblob", [1, 1280], F32R, kind="ExternalInput")
    out_d = nc.dram_tensor("out", [128, n], BF16, kind="ExternalOutput")

    with tile.TileContext(nc) as tc:
        with (
            tc.tile_pool(name="const", bufs=1) as constp,
            tc.tile_pool(name="data", bufs=1) as datap,
            tc.tile_pool(name="tbuf", bufs=1) as tbufp,
            tc.tile_pool(name="work", bufs=1) as workp,
            tc.tile_pool(name="ostage", bufs=4) as ostagep,
            tc.tile_pool(name="gacc", bufs=1, space="PSUM") as gaccp,
            tc.tile_pool(name="ppb", bufs=nppb, space="PSUM") as ppbp,
            tc.tile_pool(name="pp", bufs=3, space="PSUM") as ppp,
        ):
            # ---- constants ----
            bblob = constp.tile([128, 128], FP8, tag="bblob")
            nc.sync.dma_start(bblob, bblob_d[:, :])
            fblob = constp.tile([128, 1672], F32R, tag="fblob")
            rblob = constp.tile([1, 1280], F32R, tag="rblob")
            ident8_sb = bblob[:, 0:128]
            v1w_sb = fblob[:, 0:128]        # 128 * v1_w
            v2w_sb = fblob[:, 128:256]      # 128 * v2_w
            wT_top = fblob[:, 256:512]
            wT_bot = fblob[:, 512:768]
            TWt_k = [fblob[:, 768:1024], fblob[:, 1024:1280]]
            t2wt_k = [fblob[:, 1280:1408], fblob[:, 1408:1536]]
            v2b_pair = fblob[:, 1536:1538]   # [v2b | 0]
            v1b_pair = fblob[:, 1538:1540]   # [v1b | 0]
            t1b_sb = fblob[:, 1540:1542]
            identf_sb = fblob[:, 1544:1672]
            t2b_row_sb = rblob[:, 0:128]
            ones_row_sb = rblob[:, 128:256]
            c_row_sb = rblob[:, 256:512]
            cn_row_sb = rblob[:, 512:768]
            qb_row_sb = rblob[:, 768:896]
            kb_row_sb = rblob[:, 896:1024]
            t1c_row_sb = rblob[:, 1024:1280]  # t1w @ c / s  [1,256]

            # ---- T-pair tiles (ones cols via memset, no DMA) ----
            # PAIR tiles: two groups each. inner extent 272: DR k-tile
            # stride must be a multiple of 16
            TPs = [
                tbufp.tile([128, 2, 2, 272], FP8, tag=f"TP{i}", name=f"TP{i}")
                for i in range(ntp)
            ]
            for i, tp in enumerate(TPs):
                eng = (nc.vector, nc.gpsimd)[i % 2]
                eng.memset(tp[:, :, :, 256:257], 1.0)
                eng.memset(tp[:, :, :, 257:258], 0.0)

            for rep in range(repeat):
                # single PSUM bank holds both accumulators (258 + 130
                # cols); declared full-bank so phase 2 can reuse the slot
                # as a 4th psum buffer. Re-allocated per rep so the slot
                # rotation stays consistent across repeats.
                G_acc = gaccp.tile([128, 512], F32, tag="gacc",
                                   name=f"gacc_{rep}")
                G_top = G_acc[:, 0:258]
                G_bot = G_acc[:, 262:392]
                # ---- hi segments first (gram gates everything) ----
                hsegs = [
                    datap.tile([128, 2, hsizes[s]], FP8,
                               tag=f"hseg{s}", name=f"hseg{s}_{rep}")
                    for s in range(len(hsizes))
                ]
                lsegs = [
                    datap.tile([128, 2, lsizes[s]], FP8,
                               tag=f"lseg{s}", name=f"lseg{s}_{rep}")
                    for s in range(len(lsizes))
                ]
                for s in range(len(hsizes)):
                    nc.sync.dma_start(
                        hsegs[s], hb_d[:, 2 * hstarts[s] : 2 * hstarts[s + 1]]
                    )
                if rep == 0:
                    nc.sync.dma_start(rblob, rblob_d[:, :])
                    nc.sync.dma_start(fblob, fblob_d[:, :])
                for s in range(len(lsizes)):
                    nc.sync.dma_start(
                        lsegs[s], lb_d[:, 2 * lstarts[s] : 2 * lstarts[s + 1]]
                    )

                def hyx(col, w=128):
                    s, off = locate(col, hstarts, len(hsizes))
                    zt = hsegs[s]
                    return zt[:, 0, off : off + w], zt[:, 1, off : off + w]

                # ---- phase 1: Gram accumulation (2 chunks per group) ----
                n_groups = n_chunks // 2

                ppb_cur = [None]

                def emit_produce(h):
                    # fp8 transpose writes with element step 2 (HW
                    # constraint). Two groups share one PSUM bank; ONE
                    # copy moves the whole pair (amortizes the PSUM-access
                    # overhead below the PE pace).
                    if h % 2 == 0:
                        ppb_cur[0] = ppbp.tile(
                            [128, 2048], FP8, tag="ppb",
                            name=f"pp{(h // 2) % 8}_{rep}",
                        )
                    pp_t = ppb_cur[0]
                    base = (h % 2) * 1024
                    pp3 = pp_t.rearrange("p (c j) -> p c j", j=2)
                    for k in range(2):
                        g = 2 * h + k
                        yc, xc = hyx(g * 128)
                        off = base // 2 + 256 * k
                        nc.tensor.transpose(
                            pp3[:, off : off + 128, 0:1], yc, ident8_sb
                        )
                        nc.tensor.transpose(
                            pp3[:, off + 128 : off + 256, 0:1], xc, ident8_sb
                        )
                    if h % 2 == 1:
                        TP = TPs[(h // 2) % ntp]
                        csrc = pp_t.rearrange(
                            "p (a b c j) -> p a b c j", a=2, b=2, j=2
                        )[:, :, :, :, 0:1]
                        if (h // 2) % 2 == 0:
                            nc.vector.tensor_copy(TP[:, :, :, 0:256], csrc)
                        else:
                            nc.scalar.activation(TP[:, :, :, 0:256], csrc, AF.Copy)

                def emit_gram(h):
                    TP = TPs[(h // 2) % ntp][:, h % 2]
                    # start=True zeroes the WHOLE 2KB PSUM bank (pending-
                    # zero region), so only the very first matmul may set
                    # it; the other h==0 matmuls land on pending-zero bytes
                    # and write fresh. Ifmap streams are kept <=512 elems.
                    nc.tensor.matmul(
                        G_top[:, 0:256], TP[:, :, 0:128], TP[:, :, 0:256],
                        start=(h == 0), stop=False, perf_mode=DR,
                        skip_group_check=True,
                    )
                    nc.tensor.matmul(
                        G_top[:, 256:258], TP[:, :, 0:128], TP[:, :, 256:258],
                        start=False, stop=False, perf_mode=DR,
                        skip_group_check=True,
                    )
                    nc.tensor.matmul(
                        G_bot, TP[:, :, 128:256], TP[:, :, 128:258],
                        start=False, stop=False, perf_mode=DR,
                        skip_group_check=True,
                    )

                for h in range(n_groups + lag):
                    if h < n_groups:
                        emit_produce(h)
                    if h >= lag and not skip_gram:
                        emit_gram(h - lag)
                if skip_gram:
                    emit_gram(n_groups - 1)

                # ---- postlude ----
                # S = [yh;xh;1]-gram; col 256 = [ysum; xsum].
                # energy = W S W^T + v c^T + c (v + n c)^T,
                #   W = blkdiag(q_w, k_w), v = W [ysum;xsum], c = [q_b;k_b].
                zs_top = workp.tile([128, 1], F32R, tag="zst")
                nc.vector.tensor_copy(zs_top, G_top[:, 256:257])
                zs_bot = workp.tile([128, 1], F32R, tag="zsb")
                nc.vector.tensor_copy(zs_bot, G_bot[:, 128:129])
                S_top_sb = workp.tile([128, 256], F32R, tag="stop")
                nc.vector.tensor_copy(S_top_sb, G_top[:, 0:256])
                S_bot_sb = workp.tile([128, 256], F32R, tag="sbot")
                nc.scalar.activation(S_bot_sb[:, 128:256], G_bot[:, 0:128], AF.Copy)
                tr_ps = ppp.tile([128, 128], F32R, tag="pp")
                nc.tensor.transpose(tr_ps, S_top_sb[:, 128:256], identf_sb)
                nc.vector.tensor_copy(S_bot_sb[:, 0:128], tr_ps)

                # v row and u2 = v + n c
                v_ps = ppp.tile([1, 256], F32, tag="pp")
                nc.tensor.matmul(v_ps, zs_top, wT_top, start=True, stop=False)
                nc.tensor.matmul(v_ps, zs_bot, wT_bot, start=False, stop=True)
                v_sb = workp.tile([1, 256], F32R, tag="vsb")
                nc.scalar.activation(v_sb, v_ps, AF.Copy)
                u2_row = workp.tile([1, 256], F32R, tag="urow")
                nc.vector.tensor_add(u2_row, v_sb, cn_row_sb)
                # t1v row = (t1w W / s) zsums
                t1v_ps = ppp.tile([1, 256], F32, tag="pp")
                nc.tensor.matmul(t1v_ps, zs_top, TWt_k[0], start=True, stop=False)
                nc.tensor.matmul(t1v_ps, zs_bot, TWt_k[1], start=False, stop=True)
                t1v_sb = workp.tile([1, 256], F32R, tag="t1v")
                nc.vector.tensor_copy(t1v_sb, t1v_ps)

                # U = S W^T (rows in 2 blocks)
                U_sb = []
                for kb in range(2):
                    u_ps = ppp.tile([128, 256], F32, tag="pp")
                    nc.tensor.matmul(
                        u_ps, S_top_sb[:, kb * 128 : kb * 128 + 128], wT_top,
                        start=True, stop=False,
                    )
                    nc.tensor.matmul(
                        u_ps, S_bot_sb[:, kb * 128 : kb * 128 + 128], wT_bot,
                        start=False, stop=True,
                    )
                    usb = workp.tile([128, 256], F32R, tag=f"usb{kb}")
                    if kb == 0:
                        nc.vector.tensor_copy(usb, u_ps)
                    else:
                        nc.scalar.activation(usb, u_ps, AF.Copy)
                    U_sb.append(usb)

                # ---- MLP1 with E folded in:
                # e1T_r = relu( sum_k TWt_k[:,r]^T U_k + t1v[r] (x) c
                #               + t1c[r] (x) u2 + t1b_r )
                e1T_sb = []
                for r in range(2):
                    ps = ppp.tile([128, 256], F32, tag="pp")
                    nc.tensor.matmul(
                        ps, TWt_k[0][:, r * 128 : (r + 1) * 128], U_sb[0],
                        start=True, stop=False,
                    )
                    nc.tensor.matmul(
                        ps, TWt_k[1][:, r * 128 : (r + 1) * 128], U_sb[1],
                        start=False, stop=False,
                    )
                    nc.tensor.matmul(
                        ps, t1v_sb[:, r * 128 : (r + 1) * 128], c_row_sb,
                        start=False, stop=False, skip_group_check=True,
                    )
                    nc.tensor.matmul(
                        ps, t1c_row_sb[:, r * 128 : (r + 1) * 128], u2_row,
                        start=False, stop=True, skip_group_check=True,
                    )
                    sb = workp.tile([128, 256], F32R, tag=f"e1t{r}")
                    nc.scalar.activation(sb, ps, AF.Relu, bias=t1b_sb[:, r : r + 1])
                    e1T_sb.append(sb)

                # ---- MLP layer 2 + softmax ----
                attn = []
                for r in range(2):
                    ps = ppp.tile([128, 128], F32, tag="pp")
                    nc.tensor.matmul(
                        ps, e1T_sb[0][:, r * 128 : (r + 1) * 128], t2wt_k[0],
                        start=True, stop=False,
                    )
                    nc.tensor.matmul(
                        ps, e1T_sb[1][:, r * 128 : (r + 1) * 128], t2wt_k[1],
                        start=False, stop=False,
                    )
                    nc.tensor.matmul(
                        ps, ones_row_sb, t2b_row_sb,
                        start=False, stop=True, skip_group_check=True,
                    )
                    e2 = workp.tile([128, 128], F32, tag=f"e2_{r}")
                    nc.scalar.activation(e2, ps, AF.Relu)
                    mneg = workp.tile([128, 1], F32, tag=f"mx{r}")
                    nc.vector.tensor_reduce(
                        mneg, e2, axis=mybir.AxisListType.X,
                        op=mybir.AluOpType.max, negate=True,
                    )
                    p_t = workp.tile([128, 128], F32, tag=f"pt{r}")
                    ssum = workp.tile([128, 1], F32, tag=f"sm{r}")
                    nc.scalar.activation(p_t, e2, AF.Exp, bias=mneg, accum_out=ssum)
                    rcp = workp.tile([128, 1], F32, tag=f"rc{r}")
                    nc.vector.reciprocal(rcp, ssum)
                    a_t = workp.tile([128, 128], F32R, tag=f"attn{r}")
                    nc.vector.tensor_scalar_mul(a_t, p_t, rcp)
                    attn.append(a_t)

                # ---- fold attn into v-weights; fp8 Wa/Wb/Wc prep ----
                # v{1,2}w_sb are pre-scaled by WSCALE on the host, so
                # w?_ps = WSCALE * W^T directly.
                Wfa = workp.tile([128, 2, 128], FP8, tag="wfa")
                Wfb = workp.tile([128, 2, 128], FP8, tag="wfb")
                Wfc = workp.tile([128, 2, 128], FP8, tag="wfc")
                w_ps = []
                for p, (wsb, at) in enumerate([(v2w_sb, attn[0]), (v1w_sb, attn[1])]):
                    ps = ppp.tile([128, 128], F32, tag="pp", name=f"wf{p}_{rep}")
                    nc.tensor.matmul(ps, wsb, at, start=True, stop=True)
                    w_ps.append(ps)
                    nc.scalar.activation(Wfa[:, p, :], ps, AF.Copy)
                    nc.vector.tensor_scalar_mul(Wfb[:, p, :], ps, 1.0 / 16.0)
                for p in range(2):
                    nc.vector.tensor_tensor(
                        Wfc[:, p, :], w_ps[p], Wfa[:, p, :],
                        op=mybir.AluOpType.subtract,
                    )

                bout_ps = ppp.tile([128, 2], F32, tag="pp")
                nc.tensor.matmul(bout_ps, attn[0], v2b_pair, start=True, stop=False)
                nc.tensor.matmul(bout_ps, attn[1], v1b_pair, start=False, stop=True)
                bout_sb = workp.tile([128, 1], F32, tag="bout")
                nc.vector.tensor_copy(bout_sb, bout_ps[:, 0:1])

                # ---- phase 2: out = (Wa zh + Wb zl + Wc zh)/WSCALE + bout ----
                assert out_chunks % 4 == 0
                ot = None
                # phase-2 PSUM rotates through ALL banks: pp (3) + the
                # phase-1 staging banks (4) + the gram bank (1), all idle now
                for j in range(out_chunks if not skip_phase2 else 4):
                    hs, hoff = locate(j * oc, hstarts, len(hsizes))
                    ls, loff = locate(j * oc, lstarts, len(lsizes))
                    hz = hsegs[hs][:, :, hoff : hoff + oc]
                    lz = lsegs[ls][:, :, loff : loff + oc]
                    # 4-deep psum rotation: 3 pp banks + the retired gram bank
                    if j % 4 == 3:
                        ps = gaccp.tile([128, 512], F32, tag="gacc",
                                        name=f"opsg{(j // 4) % 2}_{rep}")
                    else:
                        ps = ppp.tile([128, 512], F32, tag="pp",
                                      name=f"ops{j % 4}_{rep}")
                    nc.tensor.matmul(ps, Wfa, hz, start=True, stop=False,
                                     perf_mode=DR)
                    nc.tensor.matmul(ps, Wfb, lz, start=False, stop=False,
                                     perf_mode=DR, skip_group_check=True)
                    nc.tensor.matmul(ps, Wfc, hz, start=False, stop=True,
                                     perf_mode=DR, skip_group_check=True)
                    if j % 4 == 0:
                        ot = ostagep.tile([128, 4 * 512], BF16, tag="ot")
                    half = ot[:, (j % 4) * oc : (j % 4) * oc + oc]
                    if j % 2 == 0:
                        nc.vector.tensor_scalar(
                            half, ps, 1.0 / WSCALE, bout_sb,
                            op0=mybir.AluOpType.mult, op1=mybir.AluOpType.add,
                        )
                    else:
                        nc.scalar.activation(
                            half, ps, AF.Identity, bias=bout_sb,
                            scale=1.0 / WSCALE,
                        )
                    if j % 4 == 3:
                        nc.sync.dma_start(
                            out_d[:, (j - 3) * oc : (j + 1) * oc], ot[:, 0 : 4 * oc]
                        )

    nc.finalize()
    return nc


_PROGRAM_CACHE = {}


def get_program(n=N):
    if n not in _PROGRAM_CACHE:
        _PROGRAM_CACHE[n] = build_program(n)
    return _PROGRAM_CACHE[n]


def prep_in_maps(inputs, n=N):
    """Host-side prep: shard over batch, hi/lo split, fold weights."""
    f8 = ml_dtypes.float8_e4m3
    f32 = np.float32
    x, y = np.asarray(inputs["x"]), np.asarray(inputs["y"])
    qw, qb = np.asarray(inputs["q_w"]), np.asarray(inputs["q_b"])
    kw, kb = np.asarray(inputs["k_w"]), np.asarray(inputs["k_b"])
    v1w, v1b = np.asarray(inputs["v1_w"]), np.asarray(inputs["v1_b"])
    v2w, v2b = np.asarray(inputs["v2_w"]), np.asarray(inputs["v2_b"])
    t1w, t1b = np.asarray(inputs["t1_w"]), np.asarray(inputs["t1_b"])
    t2w, t2b = np.asarray(inputs["t2_w"]), np.asarray(inputs["t2_b"])

    s = np.sqrt(f32(2 * C))
    cvec = np.concatenate([qb, kb]).astype(f32)
    Wblk = np.zeros((2 * C, 2 * C), f32)
    Wblk[:C, :C] = qw
    Wblk[C:, C:] = kw
    TW = (t1w.astype(f32) @ Wblk) / s          # [256, 256]
    TWt = np.ascontiguousarray(TW.T)           # [256, 256]
    t1c = (t1w.astype(f32) @ cvec) / s         # [256]
    t2wt = np.ascontiguousarray(t2w.T).astype(f32)       # [256, 128]
    z128 = np.zeros((128, 128), f32)
    wT_top = np.concatenate([qw.T.astype(f32), z128], axis=1)   # [128, 256]
    wT_bot = np.concatenate([z128, kw.T.astype(f32)], axis=1)
    fblob = np.concatenate(
        [
            WSCALE * v1w.astype(f32),                    # 0:128
            WSCALE * v2w.astype(f32),                    # 128:256
            wT_top,                                      # 256:512
            wT_bot,                                      # 512:768
            TWt[0:128, :],                               # 768:1024
            TWt[128:256, :],                             # 1024:1280
            t2wt[0:128, :],                              # 1280:1408
            t2wt[128:256, :],                            # 1408:1536
            v2b.reshape(128, 1).astype(f32),             # 1536 (pair w/ 0)
            np.zeros((128, 1), f32),                     # 1537 pad
            v1b.reshape(128, 1).astype(f32),             # 1538 (pair w/ 0)
            np.zeros((128, 1), f32),                     # 1539 pad
            t1b[0:128].reshape(128, 1).astype(f32),      # 1540
            t1b[128:256].reshape(128, 1).astype(f32),    # 1541
            np.zeros((128, 2), f32),                     # 1542 pad
            np.eye(128, dtype=f32),                      # 1544:1672
        ],
        axis=1,
    )
    rblob = np.concatenate(
        [
            t2b.astype(f32),                             # 0:128
            np.ones(128, f32),                           # 128:256
            cvec,                                        # 256:512
            f32(n) * cvec,                               # 512:768
            qb.astype(f32),                              # 768:896
            kb.astype(f32),                              # 896:1024
            t1c,                                         # 1024:1280
        ]
    ).reshape(1, 1280)
    shared = {
        "bblob": np.eye(128, dtype=f32).astype(f8),
        "fblob": np.ascontiguousarray(fblob),
        "rblob": np.ascontiguousarray(rblob),
    }

    def packed(zh_y, zh_x, sizes):
        starts = np.concatenate([[0], np.cumsum(sizes)]).astype(int)
        zb = np.empty((128, 2 * n), f8)
        for si, sz in enumerate(sizes):
            s0 = int(starts[si])
            zb[:, 2 * s0 : 2 * s0 + sz] = zh_y[:, s0 : s0 + sz]
            zb[:, 2 * s0 + sz : 2 * s0 + 2 * sz] = zh_x[:, s0 : s0 + sz]
        return zb

    hs, ls = _seg_sizes_h(n), _seg_sizes_l(n)
    in_maps = []
    for b in range(B):
        yb = np.asarray(y[b, :, :n], f32)
        xb = np.asarray(x[b, :, :n], f32)
        yh = yb.astype(f8)
        xh = xb.astype(f8)
        yl = (16.0 * (yb - yh.astype(f32))).astype(f8)
        xl = (16.0 * (xb - xh.astype(f32))).astype(f8)
        m = dict(shared)
        m["hb"] = packed(yh, xh, hs)
        m["lb"] = packed(yl, xl, ls)
        in_maps.append(m)
    return in_maps


def kernel(**inputs) -> np.ndarray:
    nc = get_program()
    in_maps = prep_in_maps(inputs)
    res = run_bass_kernel_spmd(nc, in_maps, core_ids=list(range(B)))
    return np.stack([res.results[b]["out"] for b in range(B)]).astype(np.float32)


# revision 46
# speedup vs baseline: 1.0031x; 1.0031x over previous
"""Trainium2 Bass kernel for nn_Attention_68685116997866.

Math (per batch b; C=128, N=32768):
    A = q_w @ y + q_b,  K = k_w @ x + k_b          (pointwise convs)
    energy = [A;K] @ [A;K]^T / sqrt(2C)            ([256,256] Gram)
    e1 = relu(energy @ t1_w^T + t1_b)
    e2 = relu(e1 @ t2_w^T + t2_b)
    attn = softmax(e2, axis=-1)                    ([256,128])
    out  = (attn_top^T @ v2_w) @ y + (attn_bot^T @ v1_w) @ x
         + (attn_top^T @ v2_b + attn_bot^T @ v1_b) 1^T

Strategy: data-parallel over B across 8 cores (1 batch/core), no
collectives. Inputs ship as fp8-e4m3 RESIDUAL PAIRS — zh = e4m3(z),
zl = e4m3(16*(z - zh)) — the same 2 bytes/elem as bf16 (phase 2 needs
2-byte fidelity; pure-fp8 z there fails the 2e-2 gate at 2.9e-2), but
split into two separate streams so each phase reads only what it needs:

  Phase 1 (Gram) starts as soon as the hi stream flows: fp8
  PE-transposes (output elem-step 2, a HW rule) into per-pair PSUM
  banks, one DVE/ACT-alternated copy per 2 groups into fp8 T-pair
  tiles [128,2,2,272] (DR k-tile stride must be 16-aligned; GPSIMD
  cannot read PSUM so Pool only handles SBUF memsets), then fp8
  DoubleRow matmuls (K=256, 0.5 cyc/col): S_top full width, S_bot
  right half only (symmetry), ones columns for the col-sums. Only the
  first matmul of the epoch sets start=True: start zeroes the whole
  2KB PSUM bank. The lo stream DMAs during phase-1 compute.

  Postlude: energy = W S W^T + v c^T + c(v + n c)^T via U = S W^T;
  the E stage is folded into MLP1 on the host (TWt = (t1w W / s)^T)
  with rank-1 bias terms joining MLP1's PSUM accumulation. Postlude
  matmuls are f32r (1 cyc/col at >=256 moving cols; even col offsets
  and even free sizes required).

  Phase 2 = three DoubleRow matmuls per 512-col chunk, scale-matched
  into one PSUM group: Wa*zh + Wb*zl + Wc*zh with Wa = e4m3(128 W),
  Wb = e4m3(8 W), Wc = e4m3(128 W - Wa) (e4m3 max 240: 128*W is safe,
  larger scales overflow when softmax concentrates). The stage op
  applies *(1/128) + bout and out-DMAs are merged into 2048-col quads
  (the HWDGE issue port costs ~650ns per DMA). PSUM rotates through
  the 3 pp banks plus the retired gram bank.

Segments pack [y | x] per segment so each needs ONE DMA; hi segments
ramp 512->4096 then back down so the gram tail is short.

rel err 1.48e-2 vs fp32 reference (gram-fp8 attn shift 1.38e-2 is the
dominant term; hi/lo phase 2 and bf16 out add the rest).
"""

import sys

for _p in ("/opt/trn_rl_repo",):
    if _p not in sys.path:
        sys.path.insert(0, _p)

import numpy as np
import ml_dtypes

import concourse.bass as bass  # noqa: F401
import concourse.mybir as mybir
import concourse.tile as tile
from concourse import bacc
from concourse.bass_utils import run_bass_kernel_spmd

B, C, N = 8, 128, 32768
F32 = mybir.dt.float32
F32R = mybir.dt.float32r
BF16 = mybir.dt.bfloat16
FP8 = mybir.dt.float8e4
AF = mybir.ActivationFunctionType
DR = mybir.MatmulPerfMode.DoubleRow

WSCALE = 128.0  # Wa scale; Wb = WSCALE/16, lo-plane carries 16*(z-zh)


def _seg_sizes_h(n):
    if n == N:
        return [512, 1024, 2048, 4096, 4096, 4096, 4096, 4096, 4096,
                2048, 1024, 1024, 512]
    segs = 4
    return [n // segs] * segs


def _seg_sizes_l(n):
    if n == N:
        return [4096, 4096, 4096, 4096, 4096, 4096, 4096, 2048, 1024,
                512, 512]
    segs = 4
    return [n // segs] * segs


def build_program(n=N, repeat=1, lag=7, ntp=8, nppb=4,
                  skip_phase2=False, skip_gram=False):
    """Build the per-core Bass program (one batch per core)."""
    nc = bacc.Bacc(None, target_bir_lowering=False)
    hsizes = _seg_sizes_h(n)
    lsizes = _seg_sizes_l(n)
    hstarts = np.concatenate([[0], np.cumsum(hsizes)]).tolist()
    lstarts = np.concatenate([[0], np.cumsum(lsizes)]).tolist()
    n_chunks = n // 128
    assert n_chunks % 2 == 0

    def locate(col, starts, nseg):
        for s in range(nseg):
            if col < starts[s + 1]:
                return s, col - starts[s]
        raise AssertionError(col)

    oc = 512
    out_chunks = n // oc

    # ---- DRAM I/O ----
    # hb/lb pack y and x per segment: [yseg_s | xseg_s] blocks.
    hb_d = nc.dram_tensor("hb", [128, 2 * n], FP8, kind="ExternalInput")
    lb_d = nc.dram_tensor("lb", [128, 2 * n], FP8, kind="ExternalInput")
    bblob_d = nc.dram_tensor("bblob", [128, 128], FP8, kind="ExternalInput")
    fblob_d = nc.dram_tensor("fblob", [128, 1672], F32R, kind="ExternalInput")
    rblob_d = nc.dram_tensor("rblob", [1, 1280], F32R, kind="ExternalInput")
    out_d = nc.dram_tensor("out", [128, n], BF16, kind="ExternalOutput")

    with tile.TileContext(nc) as tc:
        with (
            tc.tile_pool(name="const", bufs=1) as constp,
            tc.tile_pool(name="data", bufs=1) as datap,
            tc.tile_pool(name="tbuf", bufs=1) as tbufp,
            tc.tile_pool(name="work", bufs=1) as workp,
            tc.tile_pool(name="ostage", bufs=4) as ostagep,
            tc.tile_pool(name="gacc", bufs=1, space="PSUM") as gaccp,
            tc.tile_pool(name="ppb", bufs=nppb, space="PSUM") as ppbp,
            tc.tile_pool(name="pp", bufs=3, space="PSUM") as ppp,
        ):
            # ---- constants ----
            bblob = constp.tile([128, 128], FP8, tag="bblob")
            nc.sync.dma_start(bblob, bblob_d[:, :])
            fblob = constp.tile([128, 1672], F32R, tag="fblob")
            rblob = constp.tile([1, 1280], F32R, tag="rblob")
            ident8_sb = bblob[:, 0:128]
            v1w_sb = fblob[:, 0:128]        # 128 * v1_w
            v2w_sb = fblob[:, 128:256]      # 128 * v2_w
            wT_top = fblob[:, 256:512]
            wT_bot = fblob[:, 512:768]
            TWt_k = [fblob[:, 768:1024], fblob[:, 1024:1280]]
            t2wt_k = [fblob[:, 1280:1408], fblob[:, 1408:1536]]
            v2b_pair = fblob[:, 1536:1538]   # [v2b | 0]
            v1b_pair = fblob[:, 1538:1540]   # [v1b | 0]
            t1b_sb = fblob[:, 1540:1542]
            identf_sb = fblob[:, 1544:1672]
            t2b_row_sb = rblob[:, 0:128]
            ones_row_sb = rblob[:, 128:256]
            c_row_sb = rblob[:, 256:512]
            cn_row_sb = rblob[:, 512:768]
            qb_row_sb = rblob[:, 768:896]
            kb_row_sb = rblob[:, 896:1024]
            t1c_row_sb = rblob[:, 1024:1280]  # t1w @ c / s  [1,256]

            # ---- T-pair tiles (ones cols via memset, no DMA) ----
            # PAIR tiles: two groups each. inner extent 272: DR k-tile
            # stride must be a multiple of 16
            TPs = [
                tbufp.tile([128, 2, 2, 272], FP8, tag=f"TP{i}", name=f"TP{i}")
                for i in range(ntp)
            ]
            for i, tp in enumerate(TPs):
                eng = (nc.vector, nc.gpsimd)[i % 2]
                eng.memset(tp[:, :, :, 256:257], 1.0)
                eng.memset(tp[:, :, :, 257:258], 0.0)

            for rep in range(repeat):
                # single PSUM bank holds both accumulators (258 + 130
                # cols); declared full-bank so phase 2 can reuse the slot
                # as a 4th psum buffer. Re-allocated per rep so the slot
                # rotation stays consistent across repeats.
                G_acc = gaccp.tile([128, 512], F32, tag="gacc",
                                   name=f"gacc_{rep}")
                G_top = G_acc[:, 0:258]
                G_bot = G_acc[:, 262:392]
                # ---- hi segments first (gram gates everything) ----
                hsegs = [
                    datap.tile([128, 2, hsizes[s]], FP8,
                               tag=f"hseg{s}", name=f"hseg{s}_{rep}")
                    for s in range(len(hsizes))
                ]
                lsegs = [
                    datap.tile([128, 2, lsizes[s]], FP8,
                               tag=f"lseg{s}", name=f"lseg{s}_{rep}")
                    for s in range(len(lsizes))
                ]
                for s in range(len(hsizes)):
                    nc.sync.dma_start(
                        hsegs[s], hb_d[:, 2 * hstarts[s] : 2 * hstarts[s + 1]]
                    )
                if rep == 0:
                    nc.sync.dma_start(rblob, rblob_d[:, :])
                    nc.sync.dma_start(fblob, fblob_d[:, :])
                for s in range(len(lsizes)):
                    nc.sync.dma_start(
                        lsegs[s], lb_d[:, 2 * lstarts[s] : 2 * lstarts[s + 1]]
                    )

                def hyx(col, w=128):
                    s, off = locate(col, hstarts, len(hsizes))
                    zt = hsegs[s]
                    return zt[:, 0, off : off + w], zt[:, 1, off : off + w]

                # ---- phase 1: Gram accumulation (2 chunks per group) ----
                n_groups = n_chunks // 2

                ppb_cur = [None]

                def emit_produce(h):
                    # fp8 transpose writes with element step 2 (HW
                    # constraint). Two groups share one PSUM bank; ONE
                    # copy moves the whole pair (amortizes the PSUM-access
                    # overhead below the PE pace).
                    if h % 2 == 0:
                        ppb_cur[0] = ppbp.tile(
                            [128, 2048], FP8, tag="ppb",
                            name=f"pp{(h // 2) % 8}_{rep}",
                        )
                    pp_t = ppb_cur[0]
                    base = (h % 2) * 1024
                    pp3 = pp_t.rearrange("p (c j) -> p c j", j=2)
                    for k in range(2):
                        g = 2 * h + k
                        yc, xc = hyx(g * 128)
                        off = base // 2 + 256 * k
                        nc.tensor.transpose(
                            pp3[:, off : off + 128, 0:1], yc, ident8_sb
                        )
                        nc.tensor.transpose(
                            pp3[:, off + 128 : off + 256, 0:1], xc, ident8_sb
                        )
                    if h % 2 == 1:
                        TP = TPs[(h // 2) % ntp]
                        csrc = pp_t.rearrange(
                            "p (a b c j) -> p a b c j", a=2, b=2, j=2
                        )[:, :, :, :, 0:1]
                        if (h // 2) % 2 == 0:
                            nc.vector.tensor_copy(TP[:, :, :, 0:256], csrc)
                        else:
                            nc.scalar.activation(TP[:, :, :, 0:256], csrc, AF.Copy)

                def emit_gram(h):
                    TP = TPs[(h // 2) % ntp][:, h % 2]
                    # start=True zeroes the WHOLE 2KB PSUM bank (pending-
                    # zero region), so only the very first matmul may set
                    # it; the other h==0 matmuls land on pending-zero bytes
                    # and write fresh. Ifmap streams are kept <=512 elems.
                    nc.tensor.matmul(
                        G_top[:, 0:256], TP[:, :, 0:128], TP[:, :, 0:256],
                        start=(h == 0), stop=False, perf_mode=DR,
                        skip_group_check=True,
                    )
                    nc.tensor.matmul(
                        G_top[:, 256:258], TP[:, :, 0:128], TP[:, :, 256:258],
                        start=False, stop=False, perf_mode=DR,
                        skip_group_check=True,
                    )
                    nc.tensor.matmul(
                        G_bot, TP[:, :, 128:256], TP[:, :, 128:258],
                        start=False, stop=False, perf_mode=DR,
                        skip_group_check=True,
                    )

                for h in range(n_groups + lag):
                    if h < n_groups:
                        emit_produce(h)
                    if h >= lag and not skip_gram:
                        emit_gram(h - lag)
                if skip_gram:
                    emit_gram(n_groups - 1)

                # ---- postlude ----
                # S = [yh;xh;1]-gram; col 256 = [ysum; xsum].
                # energy = W S W^T + v c^T + c (v + n c)^T,
                #   W = blkdiag(q_w, k_w), v = W [ysum;xsum], c = [q_b;k_b].
                zs_top = workp.tile([128, 1], F32R, tag="zst")
                nc.vector.tensor_copy(zs_top, G_top[:, 256:257])
                zs_bot = workp.tile([128, 1], F32R, tag="zsb")
                nc.vector.tensor_copy(zs_bot, G_bot[:, 128:129])
                S_top_sb = workp.tile([128, 256], F32R, tag="stop")
                nc.vector.tensor_copy(S_top_sb, G_top[:, 0:256])
                S_bot_sb = workp.tile([128, 256], F32R, tag="sbot")
                nc.scalar.activation(S_bot_sb[:, 128:256], G_bot[:, 0:128], AF.Copy)
                tr_ps = ppp.tile([128, 128], F32R, tag="pp")
                nc.tensor.transpose(tr_ps, S_top_sb[:, 128:256], identf_sb)
                nc.vector.tensor_copy(S_bot_sb[:, 0:128], tr_ps)

                # v row and u2 = v + n c
                v_ps = ppp.tile([1, 256], F32, tag="pp")
                nc.tensor.matmul(v_ps, zs_top, wT_top, start=True, stop=False)
                nc.tensor.matmul(v_ps, zs_bot, wT_bot, start=False, stop=True)
                v_sb = workp.tile([1, 256], F32R, tag="vsb")
                nc.scalar.activation(v_sb, v_ps, AF.Copy)
                u2_row = workp.tile([1, 256], F32R, tag="urow")
                nc.vector.tensor_add(u2_row, v_sb, cn_row_sb)
                # t1v row = (t1w W / s) zsums
                t1v_ps = ppp.tile([1, 256], F32, tag="pp")
                nc.tensor.matmul(t1v_ps, zs_top, TWt_k[0], start=True, stop=False)
                nc.tensor.matmul(t1v_ps, zs_bot, TWt_k[1], start=False, stop=True)
                t1v_sb = workp.tile([1, 256], F32R, tag="t1v")
                nc.vector.tensor_copy(t1v_sb, t1v_ps)

                # U = S W^T (rows in 2 blocks)
                U_sb = []
                for kb in range(2):
                    u_ps = ppp.tile([128, 256], F32, tag="pp")
                    nc.tensor.matmul(
                        u_ps, S_top_sb[:, kb * 128 : kb * 128 + 128], wT_top,
                        start=True, stop=False,
                    )
                    nc.tensor.matmul(
                        u_ps, S_bot_sb[:, kb * 128 : kb * 128 + 128], wT_bot,
                        start=False, stop=True,
                    )
                    usb = workp.tile([128, 256], F32R, tag=f"usb{kb}")
                    if kb == 0:
                        nc.vector.tensor_copy(usb, u_ps)
                    else:
                        nc.scalar.activation(usb, u_ps, AF.Copy)
                    U_sb.append(usb)

                # ---- MLP1 with E folded in:
                # e1T_r = relu( sum_k TWt_k[:,r]^T U_k + t1v[r] (x) c
                #               + t1c[r] (x) u2 + t1b_r )
                e1T_sb = []
                for r in range(2):
                    ps = ppp.tile([128, 256], F32, tag="pp")
                    nc.tensor.matmul(
                        ps, TWt_k[0][:, r * 128 : (r + 1) * 128], U_sb[0],
                        start=True, stop=False,
                    )
                    nc.tensor.matmul(
                        ps, TWt_k[1][:, r * 128 : (r + 1) * 128], U_sb[1],
                        start=False, stop=False,
                    )
                    nc.tensor.matmul(
                        ps, t1v_sb[:, r * 128 : (r + 1) * 128], c_row_sb,
                        start=False, stop=False, skip_group_check=True,
                    )
                    nc.tensor.matmul(
                        ps, t1c_row_sb[:, r * 128 : (r + 1) * 128], u2_row,
                        start=False, stop=True, skip_group_check=True,
                    )
                    sb = workp.tile([128, 256], F32R, tag=f"e1t{r}")
                    nc.scalar.activation(sb, ps, AF.Relu, bias=t1b_sb[:, r : r + 1])
                    e1T_sb.append(sb)

                # ---- MLP layer 2 + softmax ----
                attn = []
                for r in range(2):
                    ps = ppp.tile([128, 128], F32, tag="pp")
                    nc.tensor.matmul(
                        ps, e1T_sb[0][:, r * 128 : (r + 1) * 128], t2wt_k[0],
                        start=True, stop=False,
                    )
                    nc.tensor.matmul(
                        ps, e1T_sb[1][:, r * 128 : (r + 1) * 128], t2wt_k[1],
                        start=False, stop=False,
                    )
                    nc.tensor.matmul(
                        ps, ones_row_sb, t2b_row_sb,
                        start=False, stop=True, skip_group_check=True,
                    )
                    e2 = workp.tile([128, 128], F32, tag=f"e2_{r}")
                    nc.scalar.activation(e2, ps, AF.Relu)
                    mneg = workp.tile([128, 1], F32, tag=f"mx{r}")
                    nc.vector.tensor_reduce(
                        mneg, e2, axis=mybir.AxisListType.X,
                        op=mybir.AluOpType.max, negate=True,
                    )
                    p_t = workp.tile([128, 128], F32, tag=f"pt{r}")
                    ssum = workp.tile([128, 1], F32, tag=f"sm{r}")
                    nc.scalar.activation(p_t, e2, AF.Exp, bias=mneg, accum_out=ssum)
                    rcp = workp.tile([128, 1], F32, tag=f"rc{r}")
                    nc.vector.reciprocal(rcp, ssum)
                    a_t = workp.tile([128, 128], F32R, tag=f"attn{r}")
                    nc.vector.tensor_scalar_mul(a_t, p_t, rcp)
                    attn.append(a_t)

                # ---- fold attn into v-weights; fp8 Wa/Wb/Wc prep ----
                # v{1,2}w_sb are pre-scaled by WSCALE on the host, so
                # w?_ps = WSCALE * W^T directly.
                Wfa = workp.tile([128, 2, 128], FP8, tag="wfa")
                Wfb = workp.tile([128, 2, 128], FP8, tag="wfb")
                Wfc = workp.tile([128, 2, 128], FP8, tag="wfc")
                w_ps = []
                for p, (wsb, at) in enumerate([(v2w_sb, attn[0]), (v1w_sb, attn[1])]):
                    ps = ppp.tile([128, 128], F32, tag="pp", name=f"wf{p}_{rep}")
                    nc.tensor.matmul(ps, wsb, at, start=True, stop=True)
                    w_ps.append(ps)
                for p in range(2):
                    nc.scalar.activation(Wfa[:, p, :], w_ps[p], AF.Copy)
                    nc.vector.tensor_scalar_mul(Wfb[:, p, :], w_ps[p], 1.0 / 16.0)
                for p in range(2):
                    nc.vector.tensor_tensor(
                        Wfc[:, p, :], w_ps[p], Wfa[:, p, :],
                        op=mybir.AluOpType.subtract,
                    )

                bout_ps = ppp.tile([128, 2], F32, tag="pp")
                nc.tensor.matmul(bout_ps, attn[0], v2b_pair, start=True, stop=False)
                nc.tensor.matmul(bout_ps, attn[1], v1b_pair, start=False, stop=True)
                bout_sb = workp.tile([128, 1], F32, tag="bout")
                nc.vector.tensor_copy(bout_sb, bout_ps[:, 0:1])

                # ---- phase 2: out = (Wa zh + Wb zl + Wc zh)/WSCALE + bout ----
                assert out_chunks % 4 == 0
                ot = None
                # phase-2 PSUM rotates through ALL banks: pp (3) + the
                # phase-1 staging banks (4) + the gram bank (1), all idle now
                for j in range(out_chunks if not skip_phase2 else 4):
                    hs, hoff = locate(j * oc, hstarts, len(hsizes))
                    ls, loff = locate(j * oc, lstarts, len(lsizes))
                    hz = hsegs[hs][:, :, hoff : hoff + oc]
                    lz = lsegs[ls][:, :, loff : loff + oc]
                    # 4-deep psum rotation: 3 pp banks + the retired gram bank
                    if j % 4 == 3:
                        ps = gaccp.tile([128, 512], F32, tag="gacc",
                                        name=f"opsg{(j // 4) % 2}_{rep}")
                    else:
                        ps = ppp.tile([128, 512], F32, tag="pp",
                                      name=f"ops{j % 4}_{rep}")
                    nc.tensor.matmul(ps, Wfa, hz, start=True, stop=False,
                                     perf_mode=DR)
                    nc.tensor.matmul(ps, Wfb, lz, start=False, stop=False,
                                     perf_mode=DR, skip_group_check=True)
                    nc.tensor.matmul(ps, Wfc, hz, start=False, stop=True,
                                     perf_mode=DR, skip_group_check=True)
                    if j % 4 == 0:
                        ot = ostagep.tile([128, 4 * 512], BF16, tag="ot")
                    half = ot[:, (j % 4) * oc : (j % 4) * oc + oc]
                    if j % 2 == 0:
                        nc.vector.tensor_scalar(
                            half, ps, 1.0 / WSCALE, bout_sb,
                            op0=mybir.AluOpType.mult, op1=mybir.AluOpType.add,
                        )
                    else:
                        nc.scalar.activation(
                            half, ps, AF.Identity, bias=bout_sb,
                            scale=1.0 / WSCALE,
                        )
                    if j % 4 == 3:
                        nc.sync.dma_start(
                            out_d[:, (j - 3) * oc : (j + 1) * oc], ot[:, 0 : 4 * oc]
                        )

    nc.finalize()
    return nc


_PROGRAM_CACHE = {}


def get_program(n=N):
    if n not in _PROGRAM_CACHE:
        _PROGRAM_CACHE[n] = build_program(n)
    return _PROGRAM_CACHE[n]


def prep_in_maps(inputs, n=N):
    """Host-side prep: shard over batch, hi/lo split, fold weights."""
    f8 = ml_dtypes.float8_e4m3
    f32 = np.float32
    x, y = np.asarray(inputs["x"]), np.asarray(inputs["y"])
    qw, qb = np.asarray(inputs["q_w"]), np.asarray(inputs["q_b"])
    kw, kb = np.asarray(inputs["k_w"]), np.asarray(inputs["k_b"])
    v1w, v1b = np.asarray(inputs["v1_w"]), np.asarray(inputs["v1_b"])
    v2w, v2b = np.asarray(inputs["v2_w"]), np.asarray(inputs["v2_b"])
    t1w, t1b = np.asarray(inputs["t1_w"]), np.asarray(inputs["t1_b"])
    t2w, t2b = np.asarray(inputs["t2_w"]), np.asarray(inputs["t2_b"])

    s = np.sqrt(f32(2 * C))
    cvec = np.concatenate([qb, kb]).astype(f32)
    Wblk = np.zeros((2 * C, 2 * C), f32)
    Wblk[:C, :C] = qw
    Wblk[C:, C:] = kw
    TW = (t1w.astype(f32) @ Wblk) / s          # [256, 256]
    TWt = np.ascontiguousarray(TW.T)           # [256, 256]
    t1c = (t1w.astype(f32) @ cvec) / s         # [256]
    t2wt = np.ascontiguousarray(t2w.T).astype(f32)       # [256, 128]
    z128 = np.zeros((128, 128), f32)
    wT_top = np.concatenate([qw.T.astype(f32), z128], axis=1)   # [128, 256]
    wT_bot = np.concatenate([z128, kw.T.astype(f32)], axis=1)
    fblob = np.concatenate(
        [
            WSCALE * v1w.astype(f32),                    # 0:128
            WSCALE * v2w.astype(f32),                    # 128:256
            wT_top,                                      # 256:512
            wT_bot,                                      # 512:768
            TWt[0:128, :],                               # 768:1024
            TWt[128:256, :],                             # 1024:1280
            t2wt[0:128, :],                              # 1280:1408
            t2wt[128:256, :],                            # 1408:1536
            v2b.reshape(128, 1).astype(f32),             # 1536 (pair w/ 0)
            np.zeros((128, 1), f32),                     # 1537 pad
            v1b.reshape(128, 1).astype(f32),             # 1538 (pair w/ 0)
            np.zeros((128, 1), f32),                     # 1539 pad
            t1b[0:128].reshape(128, 1).astype(f32),      # 1540
            t1b[128:256].reshape(128, 1).astype(f32),    # 1541
            np.zeros((128, 2), f32),                     # 1542 pad
            np.eye(128, dtype=f32),                      # 1544:1672
        ],
        axis=1,
    )
    rblob = np.concatenate(
        [
            t2b.astype(f32),                             # 0:128
            np.ones(128, f32),                           # 128:256
            cvec,                                        # 256:512
            f32(n) * cvec,                               # 512:768
            qb.astype(f32),                              # 768:896
            kb.astype(f32),                              # 896:1024
            t1c,                                         # 1024:1280
        ]
    ).reshape(1, 1280)
    shared = {
        "bblob": np.eye(128, dtype=f32).astype(f8),
        "fblob": np.ascontiguousarray(fblob),
        "rblob": np.ascontiguousarray(rblob),
    }

    def packed(zh_y, zh_x, sizes):
        starts = np.concatenate([[0], np.cumsum(sizes)]).astype(int)
        zb = np.empty((128, 2 * n), f8)
        for si, sz in enumerate(sizes):
            s0 = int(starts[si])
            zb[:, 2 * s0 : 2 * s0 + sz] = zh_y[:, s0 : s0 + sz]
            zb[:, 2 * s0 + sz : 2 * s0 + 2 * sz] = zh_x[:, s0 : s0 + sz]
        return zb

    hs, ls = _seg_sizes_h(n), _seg_sizes_l(n)
    in_maps = []
    for b in range(B):
        yb = np.asarray(y[b, :, :n], f32)
        xb = np.asarray(x[b, :, :n], f32)
        yh = yb.astype(f8)
        xh = xb.astype(f8)
        yl = (16.0 * (yb - yh.astype(f32))).astype(f8)
        xl = (16.0 * (xb - xh.astype(f32))).astype(f8)
        m = dict(shared)
        m["hb"] = packed(yh, xh, hs)
        m["lb"] = packed(yl, xl, ls)
        in_maps.append(m)
    return in_maps


def kernel(**inputs) -> np.ndarray:
    nc = get_program()
    in_maps = prep_in_maps(inputs)
    res = run_bass_kernel_spmd(nc, in_maps, core_ids=list(range(B)))
    return np.stack([res.results[b]["out"] for b in range(B)]).astype(np.float32)


# revision 51
# speedup vs baseline: 1.0202x; 1.0170x over previous
"""Trainium2 Bass kernel for nn_Attention_68685116997866.

Math (per batch b; C=128, N=32768):
    A = q_w @ y + q_b,  K = k_w @ x + k_b          (pointwise convs)
    energy = [A;K] @ [A;K]^T / sqrt(2C)            ([256,256] Gram)
    e1 = relu(energy @ t1_w^T + t1_b)
    e2 = relu(e1 @ t2_w^T + t2_b)
    attn = softmax(e2, axis=-1)                    ([256,128])
    out  = (attn_top^T @ v2_w) @ y + (attn_bot^T @ v1_w) @ x
         + (attn_top^T @ v2_b + attn_bot^T @ v1_b) 1^T

Strategy: data-parallel over B across 8 cores (1 batch/core), no
collectives. Inputs ship as fp8-e4m3 RESIDUAL PAIRS — zh = e4m3(z),
zl = e4m3(16*(z - zh)) — the same 2 bytes/elem as bf16 (phase 2 needs
2-byte fidelity; pure-fp8 z there fails the 2e-2 gate at 2.9e-2), but
split into two separate streams so each phase reads only what it needs:

  Phase 1 (Gram) starts as soon as the hi stream flows: fp8
  PE-transposes (output elem-step 2, a HW rule) into per-pair PSUM
  banks, one DVE/ACT-alternated copy per 2 groups into fp8 T-pair
  tiles [128,2,2,272] (DR k-tile stride must be 16-aligned; GPSIMD
  cannot read PSUM so Pool only handles SBUF memsets), then fp8
  DoubleRow matmuls (K=256, 0.5 cyc/col): S_top full width, S_bot
  right half only (symmetry), ones columns for the col-sums. Only the
  first matmul of the epoch sets start=True: start zeroes the whole
  2KB PSUM bank. The lo stream DMAs during phase-1 compute.

  Postlude: energy = W S W^T + v c^T + c(v + n c)^T via U = S W^T;
  the E stage is folded into MLP1 on the host (TWt = (t1w W / s)^T)
  with rank-1 bias terms joining MLP1's PSUM accumulation. Postlude
  matmuls are f32r (1 cyc/col at >=256 moving cols; even col offsets
  and even free sizes required).

  Phase 2 = three DoubleRow matmuls per 512-col chunk, scale-matched
  into one PSUM group: Wa*zh + Wb*zl + Wc*zh with Wa = e4m3(128 W),
  Wb = e4m3(8 W), Wc = e4m3(128 W - Wa) (e4m3 max 240: 128*W is safe,
  larger scales overflow when softmax concentrates). The stage op
  applies *(1/128) + bout and out-DMAs are merged into 2048-col quads
  (the HWDGE issue port costs ~650ns per DMA). PSUM rotates through
  the 3 pp banks plus the retired gram bank.

Segments pack [y | x] per segment so each needs ONE DMA; hi segments
ramp 512->4096 then back down so the gram tail is short.

rel err 1.48e-2 vs fp32 reference (gram-fp8 attn shift 1.38e-2 is the
dominant term; hi/lo phase 2 and bf16 out add the rest).
"""

import sys

for _p in ("/opt/trn_rl_repo",):
    if _p not in sys.path:
        sys.path.insert(0, _p)

import numpy as np
import ml_dtypes

import concourse.bass as bass  # noqa: F401
import concourse.mybir as mybir
import concourse.tile as tile
from concourse import bacc
from concourse.bass_utils import run_bass_kernel_spmd

B, C, N = 8, 128, 32768
F32 = mybir.dt.float32
F32R = mybir.dt.float32r
BF16 = mybir.dt.bfloat16
FP8 = mybir.dt.float8e4
AF = mybir.ActivationFunctionType
DR = mybir.MatmulPerfMode.DoubleRow

WSCALE = 128.0  # Wa scale; Wb = WSCALE/16, lo-plane carries 16*(z-zh)


def _seg_sizes_h(n):
    if n == N:
        return [512, 1024, 2048, 4096, 4096, 4096, 4096, 4096, 4096,
                2048, 1024, 1024, 512]
    segs = 4
    return [n // segs] * segs


def _seg_sizes_l(n):
    if n == N:
        return [4096, 4096, 4096, 4096, 4096, 4096, 4096, 2048, 1024,
                512, 512]
    segs = 4
    return [n // segs] * segs


def build_program(n=N, repeat=1, lag=7, ntp=8, nppb=4,
                  skip_phase2=False, skip_gram=False):
    """Build the per-core Bass program (one batch per core)."""
    nc = bacc.Bacc(None, target_bir_lowering=False)
    hsizes = _seg_sizes_h(n)
    lsizes = _seg_sizes_l(n)
    hstarts = np.concatenate([[0], np.cumsum(hsizes)]).tolist()
    lstarts = np.concatenate([[0], np.cumsum(lsizes)]).tolist()
    n_chunks = n // 128
    assert n_chunks % 2 == 0

    def locate(col, starts, nseg):
        for s in range(nseg):
            if col < starts[s + 1]:
                return s, col - starts[s]
        raise AssertionError(col)

    oc = 512
    out_chunks = n // oc

    # ---- DRAM I/O ----
    # hb/lb pack y and x per segment: [yseg_s | xseg_s] blocks.
    hb_d = nc.dram_tensor("hb", [128, 2 * n], FP8, kind="ExternalInput")
    lb_d = nc.dram_tensor("lb", [128, 2 * n], FP8, kind="ExternalInput")
    bblob_d = nc.dram_tensor("bblob", [128, 128], FP8, kind="ExternalInput")
    fblob_d = nc.dram_tensor("fblob", [128, 1672], F32R, kind="ExternalInput")
    rblob_d = nc.dram_tensor("rblob", [1, 1280], F32R, kind="ExternalInput")
    out_d = nc.dram_tensor("out", [128, n], BF16, kind="ExternalOutput")

    with tile.TileContext(nc) as tc:
        with (
            tc.tile_pool(name="const", bufs=1) as constp,
            tc.tile_pool(name="data", bufs=1) as datap,
            tc.tile_pool(name="tbuf", bufs=1) as tbufp,
            tc.tile_pool(name="work", bufs=1) as workp,
            tc.tile_pool(name="ostage", bufs=8) as ostagep,
            tc.tile_pool(name="gacc", bufs=1, space="PSUM") as gaccp,
            tc.tile_pool(name="ppb", bufs=nppb, space="PSUM") as ppbp,
            tc.tile_pool(name="pp", bufs=3, space="PSUM") as ppp,
        ):
            # ---- constants ----
            bblob = constp.tile([128, 128], FP8, tag="bblob")
            nc.sync.dma_start(bblob, bblob_d[:, :])
            fblob = constp.tile([128, 1672], F32R, tag="fblob")
            rblob = constp.tile([1, 1280], F32R, tag="rblob")
            ident8_sb = bblob[:, 0:128]
            v1w_sb = fblob[:, 0:128]        # 128 * v1_w
            v2w_sb = fblob[:, 128:256]      # 128 * v2_w
            wT_top = fblob[:, 256:512]
            wT_bot = fblob[:, 512:768]
            TWt_k = [fblob[:, 768:1024], fblob[:, 1024:1280]]
            t2wt_k = [fblob[:, 1280:1408], fblob[:, 1408:1536]]
            v2b_pair = fblob[:, 1536:1538]   # [v2b | 0]
            v1b_pair = fblob[:, 1538:1540]   # [v1b | 0]
            t1b_sb = fblob[:, 1540:1542]
            identf_sb = fblob[:, 1544:1672]
            t2b_row_sb = rblob[:, 0:128]
            ones_row_sb = rblob[:, 128:256]
            c_row_sb = rblob[:, 256:512]
            cn_row_sb = rblob[:, 512:768]
            qb_row_sb = rblob[:, 768:896]
            kb_row_sb = rblob[:, 896:1024]
            t1c_row_sb = rblob[:, 1024:1280]  # t1w @ c / s  [1,256]

            # ---- T-pair tiles (ones cols via memset, no DMA) ----
            # PAIR tiles: two groups each. inner extent 272: DR k-tile
            # stride must be a multiple of 16
            TPs = [
                tbufp.tile([128, 2, 2, 272], FP8, tag=f"TP{i}", name=f"TP{i}")
                for i in range(ntp)
            ]
            for i, tp in enumerate(TPs):
                eng = (nc.vector, nc.gpsimd)[i % 2]
                eng.memset(tp[:, :, :, 256:257], 1.0)
                eng.memset(tp[:, :, :, 257:258], 0.0)

            for rep in range(repeat):
                # single PSUM bank holds both accumulators (258 + 130
                # cols); declared full-bank so phase 2 can reuse the slot
                # as a 4th psum buffer. Re-allocated per rep so the slot
                # rotation stays consistent across repeats.
                G_acc = gaccp.tile([128, 512], F32, tag="gacc",
                                   name=f"gacc_{rep}")
                G_top = G_acc[:, 0:258]
                G_bot = G_acc[:, 262:392]
                # ---- hi segments first (gram gates everything) ----
                hsegs = [
                    datap.tile([128, 2, hsizes[s]], FP8,
                               tag=f"hseg{s}", name=f"hseg{s}_{rep}")
                    for s in range(len(hsizes))
                ]
                lsegs = [
                    datap.tile([128, 2, lsizes[s]], FP8,
                               tag=f"lseg{s}", name=f"lseg{s}_{rep}")
                    for s in range(len(lsizes))
                ]
                for s in range(len(hsizes)):
                    nc.sync.dma_start(
                        hsegs[s], hb_d[:, 2 * hstarts[s] : 2 * hstarts[s + 1]]
                    )
                if rep == 0:
                    nc.sync.dma_start(rblob, rblob_d[:, :])
                    nc.sync.dma_start(fblob, fblob_d[:, :])
                for s in range(len(lsizes)):
                    nc.sync.dma_start(
                        lsegs[s], lb_d[:, 2 * lstarts[s] : 2 * lstarts[s + 1]]
                    )

                def hyx(col, w=128):
                    s, off = locate(col, hstarts, len(hsizes))
                    zt = hsegs[s]
                    return zt[:, 0, off : off + w], zt[:, 1, off : off + w]

                # ---- phase 1: Gram accumulation (2 chunks per group) ----
                n_groups = n_chunks // 2

                ppb_cur = [None]

                def emit_produce(h):
                    # fp8 transpose writes with element step 2 (HW
                    # constraint). Two groups share one PSUM bank; ONE
                    # copy moves the whole pair (amortizes the PSUM-access
                    # overhead below the PE pace).
                    if h % 2 == 0:
                        ppb_cur[0] = ppbp.tile(
                            [128, 2048], FP8, tag="ppb",
                            name=f"pp{(h // 2) % 8}_{rep}",
                        )
                    pp_t = ppb_cur[0]
                    base = (h % 2) * 1024
                    pp3 = pp_t.rearrange("p (c j) -> p c j", j=2)
                    for k in range(2):
                        g = 2 * h + k
                        yc, xc = hyx(g * 128)
                        off = base // 2 + 256 * k
                        nc.tensor.transpose(
                            pp3[:, off : off + 128, 0:1], yc, ident8_sb
                        )
                        nc.tensor.transpose(
                            pp3[:, off + 128 : off + 256, 0:1], xc, ident8_sb
                        )
                    if h % 2 == 1:
                        TP = TPs[(h // 2) % ntp]
                        csrc = pp_t.rearrange(
                            "p (a b c j) -> p a b c j", a=2, b=2, j=2
                        )[:, :, :, :, 0:1]
                        if (h // 2) % 2 == 0:
                            nc.vector.tensor_copy(TP[:, :, :, 0:256], csrc)
                        else:
                            nc.scalar.activation(TP[:, :, :, 0:256], csrc, AF.Copy)

                def emit_gram(h):
                    TP = TPs[(h // 2) % ntp][:, h % 2]
                    # start=True zeroes the WHOLE 2KB PSUM bank (pending-
                    # zero region), so only the very first matmul may set
                    # it; the other h==0 matmuls land on pending-zero bytes
                    # and write fresh. Ifmap streams are kept <=512 elems.
                    nc.tensor.matmul(
                        G_top[:, 0:256], TP[:, :, 0:128], TP[:, :, 0:256],
                        start=(h == 0), stop=False, perf_mode=DR,
                        skip_group_check=True,
                    )
                    nc.tensor.matmul(
                        G_top[:, 256:258], TP[:, :, 0:128], TP[:, :, 256:258],
                        start=False, stop=False, perf_mode=DR,
                        skip_group_check=True,
                    )
                    nc.tensor.matmul(
                        G_bot, TP[:, :, 128:256], TP[:, :, 128:258],
                        start=False, stop=False, perf_mode=DR,
                        skip_group_check=True,
                    )

                for h in range(n_groups + lag):
                    if h < n_groups:
                        emit_produce(h)
                    if h >= lag and not skip_gram:
                        emit_gram(h - lag)
                if skip_gram:
                    emit_gram(n_groups - 1)

                # ---- postlude ----
                # S = [yh;xh;1]-gram; col 256 = [ysum; xsum].
                # energy = W S W^T + v c^T + c (v + n c)^T,
                #   W = blkdiag(q_w, k_w), v = W [ysum;xsum], c = [q_b;k_b].
                zs_top = workp.tile([128, 1], F32R, tag="zst")
                nc.vector.tensor_copy(zs_top, G_top[:, 256:257])
                zs_bot = workp.tile([128, 1], F32R, tag="zsb")
                nc.vector.tensor_copy(zs_bot, G_bot[:, 128:129])
                S_top_sb = workp.tile([128, 256], F32R, tag="stop")
                nc.vector.tensor_copy(S_top_sb, G_top[:, 0:256])
                S_bot_sb = workp.tile([128, 256], F32R, tag="sbot")
                nc.scalar.activation(S_bot_sb[:, 128:256], G_bot[:, 0:128], AF.Copy)
                tr_ps = ppp.tile([128, 128], F32R, tag="pp")
                nc.tensor.transpose(tr_ps, S_top_sb[:, 128:256], identf_sb)
                nc.vector.tensor_copy(S_bot_sb[:, 0:128], tr_ps)

                # v row and u2 = v + n c
                v_ps = ppp.tile([1, 256], F32, tag="pp")
                nc.tensor.matmul(v_ps, zs_top, wT_top, start=True, stop=False)
                nc.tensor.matmul(v_ps, zs_bot, wT_bot, start=False, stop=True)
                v_sb = workp.tile([1, 256], F32R, tag="vsb")
                nc.scalar.activation(v_sb, v_ps, AF.Copy)
                u2_row = workp.tile([1, 256], F32R, tag="urow")
                nc.vector.tensor_add(u2_row, v_sb, cn_row_sb)
                # t1v row = (t1w W / s) zsums
                t1v_ps = ppp.tile([1, 256], F32, tag="pp")
                nc.tensor.matmul(t1v_ps, zs_top, TWt_k[0], start=True, stop=False)
                nc.tensor.matmul(t1v_ps, zs_bot, TWt_k[1], start=False, stop=True)
                t1v_sb = workp.tile([1, 256], F32R, tag="t1v")
                nc.vector.tensor_copy(t1v_sb, t1v_ps)

                # U = S W^T (rows in 2 blocks)
                U_sb = []
                for kb in range(2):
                    u_ps = ppp.tile([128, 256], F32, tag="pp")
                    nc.tensor.matmul(
                        u_ps, S_top_sb[:, kb * 128 : kb * 128 + 128], wT_top,
                        start=True, stop=False,
                    )
                    nc.tensor.matmul(
                        u_ps, S_bot_sb[:, kb * 128 : kb * 128 + 128], wT_bot,
                        start=False, stop=True,
                    )
                    usb = workp.tile([128, 256], F32R, tag=f"usb{kb}")
                    if kb == 0:
                        nc.vector.tensor_copy(usb, u_ps)
                    else:
                        nc.scalar.activation(usb, u_ps, AF.Copy)
                    U_sb.append(usb)

                # ---- MLP1 with E folded in:
                # e1T_r = relu( sum_k TWt_k[:,r]^T U_k + t1v[r] (x) c
                #               + t1c[r] (x) u2 + t1b_r )
                e1T_sb = []
                for r in range(2):
                    ps = ppp.tile([128, 256], F32, tag="pp")
                    nc.tensor.matmul(
                        ps, TWt_k[0][:, r * 128 : (r + 1) * 128], U_sb[0],
                        start=True, stop=False,
                    )
                    nc.tensor.matmul(
                        ps, TWt_k[1][:, r * 128 : (r + 1) * 128], U_sb[1],
                        start=False, stop=False,
                    )
                    nc.tensor.matmul(
                        ps, t1v_sb[:, r * 128 : (r + 1) * 128], c_row_sb,
                        start=False, stop=False, skip_group_check=True,
                    )
                    nc.tensor.matmul(
                        ps, t1c_row_sb[:, r * 128 : (r + 1) * 128], u2_row,
                        start=False, stop=True, skip_group_check=True,
                    )
                    sb = workp.tile([128, 256], F32R, tag=f"e1t{r}")
                    nc.scalar.activation(sb, ps, AF.Relu, bias=t1b_sb[:, r : r + 1])
                    e1T_sb.append(sb)

                # ---- MLP layer 2 + softmax ----
                attn = []
                for r in range(2):
                    ps = ppp.tile([128, 128], F32, tag="pp")
                    nc.tensor.matmul(
                        ps, e1T_sb[0][:, r * 128 : (r + 1) * 128], t2wt_k[0],
                        start=True, stop=False,
                    )
                    nc.tensor.matmul(
                        ps, e1T_sb[1][:, r * 128 : (r + 1) * 128], t2wt_k[1],
                        start=False, stop=False,
                    )
                    nc.tensor.matmul(
                        ps, ones_row_sb, t2b_row_sb,
                        start=False, stop=True, skip_group_check=True,
                    )
                    e2 = workp.tile([128, 128], F32, tag=f"e2_{r}")
                    nc.scalar.activation(e2, ps, AF.Relu)
                    mneg = workp.tile([128, 1], F32, tag=f"mx{r}")
                    nc.vector.tensor_reduce(
                        mneg, e2, axis=mybir.AxisListType.X,
                        op=mybir.AluOpType.max, negate=True,
                    )
                    p_t = workp.tile([128, 128], F32, tag=f"pt{r}")
                    ssum = workp.tile([128, 1], F32, tag=f"sm{r}")
                    nc.scalar.activation(p_t, e2, AF.Exp, bias=mneg, accum_out=ssum)
                    rcp = workp.tile([128, 1], F32, tag=f"rc{r}")
                    nc.vector.reciprocal(rcp, ssum)
                    a_t = workp.tile([128, 128], F32R, tag=f"attn{r}")
                    nc.vector.tensor_scalar_mul(a_t, p_t, rcp)
                    attn.append(a_t)

                # ---- fold attn into v-weights; fp8 Wa/Wb/Wc prep ----
                # v{1,2}w_sb are pre-scaled by WSCALE on the host, so
                # w?_ps = WSCALE * W^T directly.
                Wfa = workp.tile([128, 2, 128], FP8, tag="wfa")
                Wfb = workp.tile([128, 2, 128], FP8, tag="wfb")
                Wfc = workp.tile([128, 2, 128], FP8, tag="wfc")
                w_ps = []
                for p, (wsb, at) in enumerate([(v2w_sb, attn[0]), (v1w_sb, attn[1])]):
                    ps = ppp.tile([128, 128], F32, tag="pp", name=f"wf{p}_{rep}")
                    nc.tensor.matmul(ps, wsb, at, start=True, stop=True)
                    w_ps.append(ps)
                for p in range(2):
                    nc.scalar.activation(Wfa[:, p, :], w_ps[p], AF.Copy)
                    nc.vector.tensor_scalar_mul(Wfb[:, p, :], w_ps[p], 1.0 / 16.0)
                for p in range(2):
                    nc.vector.tensor_tensor(
                        Wfc[:, p, :], w_ps[p], Wfa[:, p, :],
                        op=mybir.AluOpType.subtract,
                    )

                bout_ps = ppp.tile([128, 2], F32, tag="pp")
                nc.tensor.matmul(bout_ps, attn[0], v2b_pair, start=True, stop=False)
                nc.tensor.matmul(bout_ps, attn[1], v1b_pair, start=False, stop=True)
                bout_sb = workp.tile([128, 1], F32, tag="bout")
                nc.vector.tensor_copy(bout_sb, bout_ps[:, 0:1])

                # ---- phase 2: out = (Wa zh + Wb zl + Wc zh)/WSCALE + bout ----
                assert out_chunks % 4 == 0
                ot = None
                # phase-2 PSUM rotates through ALL banks: pp (3) + the
                # phase-1 staging banks (4) + the gram bank (1), all idle now
                for j in range(out_chunks if not skip_phase2 else 4):
                    hs, hoff = locate(j * oc, hstarts, len(hsizes))
                    ls, loff = locate(j * oc, lstarts, len(lsizes))
                    hz = hsegs[hs][:, :, hoff : hoff + oc]
                    lz = lsegs[ls][:, :, loff : loff + oc]
                    # 4-deep psum rotation: 3 pp banks + the retired gram bank
                    if j % 4 == 3:
                        ps = gaccp.tile([128, 512], F32, tag="gacc",
                                        name=f"opsg{(j // 4) % 2}_{rep}")
                    else:
                        ps = ppp.tile([128, 512], F32, tag="pp",
                                      name=f"ops{j % 4}_{rep}")
                    nc.tensor.matmul(ps, Wfa, hz, start=True, stop=False,
                                     perf_mode=DR)
                    nc.tensor.matmul(ps, Wfb, lz, start=False, stop=False,
                                     perf_mode=DR, skip_group_check=True)
                    nc.tensor.matmul(ps, Wfc, hz, start=False, stop=True,
                                     perf_mode=DR, skip_group_check=True)
                    if j % 4 == 0:
                        ot = ostagep.tile([128, 4 * 512], BF16, tag="ot")
                    half = ot[:, (j % 4) * oc : (j % 4) * oc + oc]
                    if j % 2 == 0:
                        nc.vector.tensor_scalar(
                            half, ps, 1.0 / WSCALE, bout_sb,
                            op0=mybir.AluOpType.mult, op1=mybir.AluOpType.add,
                        )
                    else:
                        nc.scalar.activation(
                            half, ps, AF.Identity, bias=bout_sb,
                            scale=1.0 / WSCALE,
                        )
                    if j % 4 == 3:
                        nc.sync.dma_start(
                            out_d[:, (j - 3) * oc : (j + 1) * oc], ot[:, 0 : 4 * oc]
                        )

    nc.finalize()
    return nc


_PROGRAM_CACHE = {}


def get_program(n=N):
    if n not in _PROGRAM_CACHE:
        _PROGRAM_CACHE[n] = build_program(n)
    return _PROGRAM_CACHE[n]


def prep_in_maps(inputs, n=N):
    """Host-side prep: shard over batch, hi/lo split, fold weights."""
    f8 = ml_dtypes.float8_e4m3
    f32 = np.float32
    x, y = np.asarray(inputs["x"]), np.asarray(inputs["y"])
    qw, qb = np.asarray(inputs["q_w"]), np.asarray(inputs["q_b"])
    kw, kb = np.asarray(inputs["k_w"]), np.asarray(inputs["k_b"])
    v1w, v1b = np.asarray(inputs["v1_w"]), np.asarray(inputs["v1_b"])
    v2w, v2b = np.asarray(inputs["v2_w"]), np.asarray(inputs["v2_b"])
    t1w, t1b = np.asarray(inputs["t1_w"]), np.asarray(inputs["t1_b"])
    t2w, t2b = np.asarray(inputs["t2_w"]), np.asarray(inputs["t2_b"])

    s = np.sqrt(f32(2 * C))
    cvec = np.concatenate([qb, kb]).astype(f32)
    Wblk = np.zeros((2 * C, 2 * C), f32)
    Wblk[:C, :C] = qw
    Wblk[C:, C:] = kw
    TW = (t1w.astype(f32) @ Wblk) / s          # [256, 256]
    TWt = np.ascontiguousarray(TW.T)           # [256, 256]
    t1c = (t1w.astype(f32) @ cvec) / s         # [256]
    t2wt = np.ascontiguousarray(t2w.T).astype(f32)       # [256, 128]
    z128 = np.zeros((128, 128), f32)
    wT_top = np.concatenate([qw.T.astype(f32), z128], axis=1)   # [128, 256]
    wT_bot = np.concatenate([z128, kw.T.astype(f32)], axis=1)
    fblob = np.concatenate(
        [
            WSCALE * v1w.astype(f32),                    # 0:128
            WSCALE * v2w.astype(f32),                    # 128:256
            wT_top,                                      # 256:512
            wT_bot,                                      # 512:768
            TWt[0:128, :],                               # 768:1024
            TWt[128:256, :],                             # 1024:1280
            t2wt[0:128, :],                              # 1280:1408
            t2wt[128:256, :],                            # 1408:1536
            v2b.reshape(128, 1).astype(f32),             # 1536 (pair w/ 0)
            np.zeros((128, 1), f32),                     # 1537 pad
            v1b.reshape(128, 1).astype(f32),             # 1538 (pair w/ 0)
            np.zeros((128, 1), f32),                     # 1539 pad
            t1b[0:128].reshape(128, 1).astype(f32),      # 1540
            t1b[128:256].reshape(128, 1).astype(f32),    # 1541
            np.zeros((128, 2), f32),                     # 1542 pad
            np.eye(128, dtype=f32),                      # 1544:1672
        ],
        axis=1,
    )
    rblob = np.concatenate(
        [
            t2b.astype(f32),                             # 0:128
            np.ones(128, f32),                           # 128:256
            cvec,                                        # 256:512
            f32(n) * cvec,                               # 512:768
            qb.astype(f32),                              # 768:896
            kb.astype(f32),                              # 896:1024
            t1c,                                         # 1024:1280
        ]
    ).reshape(1, 1280)
    shared = {
        "bblob": np.eye(128, dtype=f32).astype(f8),
        "fblob": np.ascontiguousarray(fblob),
        "rblob": np.ascontiguousarray(rblob),
    }

    def packed(zh_y, zh_x, sizes):
        starts = np.concatenate([[0], np.cumsum(sizes)]).astype(int)
        zb = np.empty((128, 2 * n), f8)
        for si, sz in enumerate(sizes):
            s0 = int(starts[si])
            zb[:, 2 * s0 : 2 * s0 + sz] = zh_y[:, s0 : s0 + sz]
            zb[:, 2 * s0 + sz : 2 * s0 + 2 * sz] = zh_x[:, s0 : s0 + sz]
        return zb

    hs, ls = _seg_sizes_h(n), _seg_sizes_l(n)
    in_maps = []
    for b in range(B):
        yb = np.asarray(y[b, :, :n], f32)
        xb = np.asarray(x[b, :, :n], f32)
        yh = yb.astype(f8)
        xh = xb.astype(f8)
        yl = (16.0 * (yb - yh.astype(f32))).astype(f8)
        xl = (16.0 * (xb - xh.astype(f32))).astype(f8)
        m = dict(shared)
        m["hb"] = packed(yh, xh, hs)
        m["lb"] = packed(yl, xl, ls)
        in_maps.append(m)
    return in_maps


def kernel(**inputs) -> np.ndarray:
    nc = get_program()
    in_maps = prep_in_maps(inputs)
    res = run_bass_kernel_spmd(nc, in_maps, core_ids=list(range(B)))
    return np.stack([res.results[b]["out"] for b in range(B)]).astype(np.float32)
